# revision 18
# baseline (speedup 1.0000x reference)
"""Trainium2 Bass kernel for a ragged-length LSTM encoder.

Problem: B=64, T=2048, E=H=256 LSTM with per-row lengths; outputs [B,T,H]
(zeros past each row's length) plus final (h, c) state frozen at length-1.

Strategy (data-parallel, 8 rows per NeuronCore):
  - Everything on-chip in "transposed" layout: hidden dim on partitions,
    batch on the free dim, so elementwise gate math runs on [128, 16] tiles.
  - Recurrent matmul z_h^T = W_h^T @ h^T as 16 (2 K-tiles x 8 M-tiles)
    [128,128]x[128,8] matmuls per step with fp16 weights (fast weight load).
  - Input projection U = x @ W_x + b_eff precomputed on-device in 32-step
    chunks (fp32 matmuls off the critical path, double buffered).
  - tanh(j) = 2*sigmoid(2j) - 1 with j columns of W/b pre-scaled by 2, so a
    single Sigmoid activation covers all four gates; one Tanh for c.
  - No masking/freezing on device: rows past their length keep evolving
    harmlessly (batch rows are independent). Host zeroes output tails and
    gathers the final h from the output history and final c from a c-history
    that is DMA'd out raw.
"""

import os
import numpy as np

import concourse.bass as bass
import concourse.mybir as mybir
import concourse.tile as tile
from concourse import bacc
from concourse.bass import ds

AF = mybir.ActivationFunctionType
ALU = mybir.AluOpType
F32 = mybir.dt.float32
F16 = mybir.dt.float16

P = 128
B_LOC = 8       # batch rows per core
E = 256         # input embedding dim
H = 256         # hidden dim
G4 = 1024       # 4*H
CH = 32         # timesteps per chunk
N_CORES = 8

_CACHE = {}


def build_nc(T):
    """Build the single-core SPMD program (same program runs on all 8 cores)."""
    NCH = T // CH           # number of chunks
    ITERS = NCH // 2        # For_i iterations (2 chunks per body)
    TPAD = T + 2 * CH       # x is padded so the next-chunk prefetch never OOBs
    UCOLS = B_LOC * CH      # 256 columns per U chunk (b-major: col = b*CH + t)

    nc = bacc.Bacc(trn_type="TRN2")

    x_pad = nc.dram_tensor("x_pad", [B_LOC, TPAD, E], F32, kind="ExternalInput")
    wx = nc.dram_tensor("wx", [2, 8, P, P], F32, kind="ExternalInput")
    wh = nc.dram_tensor("wh", [2, 8, P, P], F16, kind="ExternalInput")
    bcol = nc.dram_tensor("bcol", [8, P], F32, kind="ExternalInput")
    ident = nc.dram_tensor("ident", [P, P], F32, kind="ExternalInput")

    out = nc.dram_tensor("out", [B_LOC, T, H], F32, kind="ExternalOutput")
    c_raw = nc.dram_tensor("c_raw", [NCH, P, 2 * B_LOC * CH], F32,
                           kind="ExternalOutput")

    with tile.TileContext(nc) as tc:
        with (
            tc.tile_pool(name="persist", bufs=1) as pp,
            tc.tile_pool(name="step", bufs=3) as sp,
            tc.tile_pool(name="xs", bufs=2) as xp,
            tc.tile_pool(name="zpsum", bufs=2, space="PSUM") as zp,
            tc.tile_pool(name="upsum", bufs=2, space="PSUM") as up,
            tc.tile_pool(name="tpsum", bufs=2, space="PSUM") as tp,
        ):
            # ---- persistent tiles ----
            WXs = pp.tile([P, 2, 8, P], F32, name="WXs")
            WHs = pp.tile([P, 2, 8, P], F16, name="WHs")
            BS = pp.tile([P, 8], F32, name="BS")
            IDs = pp.tile([P, P], F32, name="IDs")
            # U cols are (t*4 + bl) within each b-group block of 128
            UA = pp.tile([P, 8, UCOLS], F32, name="UA")
            UB = pp.tile([P, 8, UCOLS], F32, name="UB")
            # c/h histories: t-major so per-step column writes are contiguous
            cTe = pp.tile([P, CH, 2, B_LOC], F32, name="cTe")
            cTo = pp.tile([P, CH, 2, B_LOC], F32, name="cTo")
            oTe = pp.tile([P, CH, 2, B_LOC], F32, name="oTe")
            oTo = pp.tile([P, CH, 2, B_LOC], F32, name="oTo")
            # double-buffered h (fp16) per chain
            hbufs = [[pp.tile([P, 2, 4], F16, name=f"h{c}{s}") for s in range(2)]
                     for c in range(2)]
            warm = pp.tile([P, 1], F32, name="warm")

            nc.sync.dma_start(WXs[:], wx[:].rearrange("kt mt k m -> k kt mt m"))
            nc.sync.dma_start(WHs[:], wh[:].rearrange("kt mt k m -> k kt mt m"))
            nc.sync.dma_start(BS[:], bcol[:].rearrange("mt k -> k mt"))
            nc.sync.dma_start(IDs[:], ident[:])
            for c in range(2):
                nc.vector.memset(hbufs[c][0][:], 0.0)
            nc.vector.memset(cTo[:], 0.0)
            nc.vector.memset(warm[:], 0.0)
            # Load the sigmoid/tanh activation table before the loop so the
            # per-iteration table-load gets hoisted.
            nc.scalar.activation(warm[:], warm[:], AF.Sigmoid)
            nc.scalar.activation(warm[:], warm[:], AF.Tanh)
            # Collapse the constant-load DMA waits so downstream PE
            # instructions don't exceed the HW sync-wait slot limit.
            tc.strict_bb_all_engine_barrier()

            # ---- helpers ----
            def dma_x_chunk(t0_sv):
                """DMA one chunk of x into two [128, 256] (t,b)-major tiles."""
                tiles = []
                for bg in range(2):
                    xn = xp.tile([P, E], F32, tag=f"xn{bg}")
                    src = x_pad[bg * 4:(bg + 1) * 4, :, :][:, ds(t0_sv, CH), :]
                    nc.sync.dma_start(xn[:], src.rearrange("b t e -> t b e"))
                    tiles.append(xn)
                return tiles

            def make_xT(xn_tiles):
                """PE-transpose x chunk tiles into xT [128(e-half), 2, 256]."""
                xT = xp.tile([P, 2, UCOLS], F32, tag="xT")
                for bg in range(2):
                    for kt in range(2):
                        ps = tp.tile([P, P], F32, tag="trps")
                        nc.tensor.transpose(
                            ps[:], xn_tiles[bg][:, kt * P:(kt + 1) * P], IDs[:])
                        nc.vector.tensor_copy(
                            xT[:, kt, bg * P:(bg + 1) * P], ps[:])
                return xT

            def u_piece(xT, U, mt):
                """One M-tile of the U-chunk matmul + bias copy."""
                psu = up.tile([P, UCOLS], F32, tag="upsum")
                nc.tensor.matmul(psu[:], WXs[:, 0, mt, :], xT[:, 0, :],
                                 start=True, stop=False)
                nc.tensor.matmul(psu[:], WXs[:, 1, mt, :], xT[:, 1, :],
                                 start=False, stop=True)
                nc.vector.tensor_scalar(
                    out=U[:, mt, :], in0=psu[:], scalar1=BS[:, mt:mt + 1],
                    scalar2=None, op0=ALU.add)

            def step(c, t, U, cT, cT_prev_buf, g_step):
                """One timestep of chain c (batch rows c*4..c*4+3)."""
                h_prev = hbufs[c][g_step % 2]
                h_new = hbufs[c][1 - g_step % 2]
                bs = slice(c * 4, c * 4 + 4)
                c_prev = (cT[:, t - 1, :, bs] if t > 0
                          else cT_prev_buf[:, CH - 1, :, bs])

                zps = zp.tile([P, 32], F32, tag=f"zps{c}")
                for mt in range(8):
                    for kt in range(2):
                        nc.tensor.matmul(
                            zps[:, mt * 4:(mt + 1) * 4],
                            WHs[:, kt, mt, :],
                            h_prev[:, kt, :],
                            start=(kt == 0), stop=(kt == 1))

                zbuf = sp.tile([P, 4, 2, 4], F32, tag=f"zbuf{c}")
                uslice = U[:, :, c * P + t * 4:c * P + t * 4 + 4] \
                    .rearrange("p (g h) b -> p g h b", h=2)
                nc.vector.tensor_tensor(
                    zbuf[:], zps[:].rearrange("p (g h b) -> p g h b", h=2, b=4),
                    uslice, ALU.add)

                sg = sp.tile([P, 4, 2, 4], F32, tag=f"sg{c}")
                nc.scalar.activation(sg[:], zbuf[:], AF.Sigmoid)
                gi, gj, gf, go = sg[:, 0], sg[:, 1], sg[:, 2], sg[:, 3]

                t1 = sp.tile([P, 2, 4], F32, tag=f"t1{c}")
                nc.vector.tensor_mul(t1[:], gi, gj)
                d = sp.tile([P, 2, 4], F32, tag=f"d{c}")
                nc.vector.scalar_tensor_tensor(
                    d[:], t1[:], 2.0, gi, ALU.mult, ALU.subtract)
                ca = sp.tile([P, 2, 4], F32, tag=f"ca{c}")
                nc.gpsimd.tensor_mul(ca[:], c_prev, gf)
                nc.vector.tensor_add(cT[:, t, :, bs], ca[:], d[:])

                th = sp.tile([P, 2, 4], F32, tag=f"th{c}")
                nc.scalar.activation(th[:], cT[:, t, :, bs], AF.Tanh)
                nc.vector.tensor_mul(h_new[:], th[:], go)
                nc.gpsimd.tensor_copy(oT_cur[0][:, t, :, bs], h_new[:])

            def out_store(oT, t0_sv):
                """Transpose the chunk's h-history and DMA to `out`."""
                for bg in range(2):
                    stg = xp.tile([P, H], F32, tag=f"stg{bg}")
                    for bl in range(4):
                        b = bg * 4 + bl
                        for half in range(2):
                            ps = tp.tile([P, P], F32, tag="trps")
                            nc.tensor.transpose(
                                ps[:CH, :], oT[:, :, half, b], IDs[:])
                            nc.vector.tensor_copy(
                                stg[bl * CH:(bl + 1) * CH,
                                    half * P:(half + 1) * P], ps[:CH, :])
                    dst = out[bg * 4:(bg + 1) * 4, :, :][:, ds(t0_sv, CH), :]
                    nc.sync.dma_start(dst, stg[:])

            oT_cur = [None]

            def run_chunk(U, cT, cT_prev, oT, extras):
                """32 steps x 2 chains, with per-step dribbled extra work."""
                oT_cur[0] = oT
                for t in range(CH):
                    for c in range(2):
                        step(c, t, U, cT, cT_prev, t)
                    for fn in extras.get(t, ()):
                        fn()

            # ---- prologue: build U for chunk 0 ----
            xt0 = dma_x_chunk(0)
            xT0 = make_xT(xt0)
            for mt in range(8):
                u_piece(xT0, UA, mt)

            # ---- main loop: one iteration = chunks (2i, 2i+1) ----
            with tc.For_i(0, ITERS, 1,
                          hint_engines=(mybir.EngineType.PE,
                                        mybir.EngineType.DVE)) as it:
                t0e = it * (2 * CH)
                t0o = it * (2 * CH) + CH
                t0n = it * (2 * CH) + 2 * CH

                # even chunk; build UB (for the odd chunk) alongside
                xt_o = dma_x_chunk(t0o)
                box = {}
                extras = {2: [lambda: box.__setitem__('xT', make_xT(xt_o))]}
                for mt in range(8):
                    extras.setdefault(4 + mt, []).append(
                        lambda mt=mt: u_piece(box['xT'], UB, mt))
                run_chunk(UA, cTe, cTo, oTe, extras)

                # odd chunk; build UA for the next iteration's even chunk,
                # and store the even chunk's outputs alongside
                xt_n = dma_x_chunk(t0n)
                box2 = {}
                extras = {2: [lambda: box2.__setitem__('xT', make_xT(xt_n))]}
                for mt in range(8):
                    extras.setdefault(4 + mt, []).append(
                        lambda mt=mt: u_piece(box2['xT'], UA, mt))
                extras.setdefault(14, []).append(lambda: out_store(oTe, t0e))
                run_chunk(UB, cTo, cTe, oTo, extras)

                out_store(oTo, t0o)
                nc.sync.dma_start(
                    c_raw[ds(it * 2, 1)],
                    cTe[:].rearrange("p t h b -> p (t h b)"))
                nc.sync.dma_start(
                    c_raw[ds(it * 2 + 1, 1)],
                    cTo[:].rearrange("p t h b -> p (t h b)"))

    return nc


def _prep_weights(W, b):
    """Host-side weight prep: split/scale/reorder into device layouts."""
    W = np.asarray(W, np.float32)
    b = np.asarray(b, np.float32)
    Wx = W[:E].copy()            # [256, 1024]
    Wh = W[E:].copy()            # [256, 1024]
    b_eff = b.copy()
    # j gate (cols 256:512) pre-scaled by 2 for tanh(j) = 2*sigmoid(2j) - 1
    Wx[:, 256:512] *= 2.0
    Wh[:, 256:512] *= 2.0
    b_eff[256:512] *= 2.0
    # f gate (cols 512:768): reference uses sigmoid(f + 1)
    b_eff[512:768] += 1.0

    # Build [2, 8, 128, 128] tile arrays: arr[kt, mt] = M[kt*128:(kt+1)*128, mt*128:(mt+1)*128]
    def to_tiles(M, dt):
        a = M.reshape(2, P, 8, P).transpose(0, 2, 1, 3)
        return np.ascontiguousarray(a).astype(dt)

    wx_t = to_tiles(Wx, np.float32)
    wh_t = to_tiles(Wh, np.float16)
    bcol = np.ascontiguousarray(b_eff.reshape(8, P)).astype(np.float32)
    return wx_t, wh_t, bcol


def kernel(input_embeddings, input_length, W, b):
    from concourse.bass_utils import run_bass_kernel_spmd

    x = np.asarray(input_embeddings, np.float32)
    lengths = np.asarray(input_length, np.int32)
    B, T, _ = x.shape
    assert B == N_CORES * B_LOC

    key = T
    if key not in _CACHE:
        nc_new = build_nc(T)
        if not nc_new.is_finalized():
            nc_new.finalize()
        _CACHE[key] = nc_new
    nc = _CACHE[key]

    wx_t, wh_t, bcol = _prep_weights(W, b)
    ident = np.eye(P, dtype=np.float32)
    TPAD = T + 2 * CH

    in_maps = []
    for k in range(N_CORES):
        xs = np.zeros((B_LOC, TPAD, E), np.float32)
        xs[:, :T] = x[k * B_LOC:(k + 1) * B_LOC]
        in_maps.append({
            "x_pad": xs, "wx": wx_t, "wh": wh_t, "bcol": bcol, "ident": ident,
        })

    trace = os.environ.get("KERNEL_PROFILE", "0") == "1"
    kw = {}
    if trace:
        try:
            import profshim
            profshim.install()
            kw["tmpdir"] = os.environ.get("KERNEL_TRACE_DIR") or None
        except Exception:
            trace = False
    try:
        res = run_bass_kernel_spmd(nc, in_maps, list(range(N_CORES)),
                                   trace=trace, **kw)
    except ModuleNotFoundError:
        res = run_bass_kernel_spmd(nc, in_maps, list(range(N_CORES)), trace=False)
    global LAST_EXEC_NS
    LAST_EXEC_NS = res.exec_time_ns

    NCH = T // CH
    outputs = np.concatenate([res.results[k]["out"] for k in range(N_CORES)], axis=0)
    h_fin = np.zeros((B, H), np.float32)
    c_fin = np.zeros((B, H), np.float32)
    for bi in range(B):
        L = int(lengths[bi])
        if L > 0:
            h_fin[bi] = outputs[bi, L - 1]
            core, bl = bi // B_LOC, bi % B_LOC
            craw = res.results[core]["c_raw"]  # [NCH, 128, 512]
            ci, tr = (L - 1) // CH, (L - 1) % CH
            for half in range(2):
                c_fin[bi, half * P:(half + 1) * P] = \
                    craw[ci, :, tr * (2 * B_LOC) + half * B_LOC + bl]
        outputs[bi, L:] = 0.0

    return outputs, (h_fin, c_fin)


LAST_EXEC_NS = None


# revision 23
# speedup vs baseline: 1.2602x; 1.2602x over previous
"""Trainium2 Bass kernel for a ragged-length LSTM encoder.

Problem: B=64, T=2048, E=H=256 LSTM with per-row lengths; outputs [B,T,H]
(zeros past each row's length) plus final (h, c) state frozen at length-1.

Strategy (data-parallel, 8 rows per NeuronCore; latency-optimized serial
scan):
  - Transposed layout on chip: hidden dim on partitions, batch on free dim.
  - Recurrent matmul z_h^T = W_h^T @ h^T as 16 [128,128]x[128,8] matmuls
    per step with fp16 weights (fast weight load). i/j/f gate tiles land in
    one PSUM tile, o in another, so sigmoid(i,j,f) can fire after 12 MMs.
  - Input projection U = x @ W_x + b_eff precomputed on-device per 32-step
    chunk from a HOST-pre-transposed x (float32r matmuls, off critical
    path, double buffered).
  - tanh(j) = 2*sigmoid(2j) - 1 with j columns of W/b pre-scaled by 2.
  - h is produced as two fp16 half-tiles so the next step's K0 matmuls
    start before the second half is ready; a dummy matmul primes the PE
    pipeline right after tanh.
  - No masking/freezing on device; h and c histories are DMA'd out raw
    (t-major) and the host transposes/assembles outputs, zeroes tails,
    and gathers final h and c.
"""

import os
import numpy as np

import concourse.bass as bass
import concourse.mybir as mybir
import concourse.tile as tile
from concourse import bacc
from concourse.bass import ds

AF = mybir.ActivationFunctionType
ALU = mybir.AluOpType
F32 = mybir.dt.float32
F32R = mybir.dt.float32r
F16 = mybir.dt.float16

P = 128
B_LOC = 8       # batch rows per core
E = 256         # input embedding dim
H = 256         # hidden dim
CH = 32         # timesteps per chunk
N_CORES = 8

_CACHE = {}


def build_nc(T, use_f32r=True):
    NCH = T // CH           # number of chunks
    ITERS = NCH // 2        # For_i iterations (2 chunks per body)
    TPAD = T + 2 * CH       # xT is padded so next-chunk prefetch never OOBs
    UCOLS = B_LOC * CH      # 256 cols per U chunk, t-major: col = t*8 + b
    XDT = F32R if use_f32r else F32

    nc = bacc.Bacc(trn_type="TRN2")

    # host-pre-transposed x: [kt(e-half), e(128), t, b]
    xT_d = nc.dram_tensor("xT", [2, P, TPAD, B_LOC], XDT, kind="ExternalInput")
    wx = nc.dram_tensor("wx", [2, 8, P, P], XDT, kind="ExternalInput")
    wh = nc.dram_tensor("wh", [2, 8, P, P], F16, kind="ExternalInput")
    bcol = nc.dram_tensor("bcol", [8, P], F32, kind="ExternalInput")

    # raw histories; host reassembles. col = t*16 + half*8 + b
    o_raw = nc.dram_tensor("o_raw", [NCH, P, 16 * CH], F32,
                           kind="ExternalOutput")
    c_raw = nc.dram_tensor("c_raw", [NCH, P, 16 * CH], F32,
                           kind="ExternalOutput")

    with tile.TileContext(nc) as tc:
        with (
            tc.tile_pool(name="persist", bufs=1) as pp,
            tc.tile_pool(name="step", bufs=3) as sp,
            tc.tile_pool(name="xs", bufs=2) as xp,
            tc.tile_pool(name="zpsum", bufs=2, space="PSUM") as zp,
            tc.tile_pool(name="upsum", bufs=2, space="PSUM") as up,
        ):
            # ---- persistent tiles ----
            WXs = pp.tile([P, 2, 8, P], XDT, name="WXs")
            WHs = pp.tile([P, 2, 8, P], F16, name="WHs")
            BS = pp.tile([P, 8], F32, name="BS")
            UA = pp.tile([P, 8, UCOLS], F32, name="UA")
            UB = pp.tile([P, 8, UCOLS], F32, name="UB")
            cTe = pp.tile([P, CH, 16], F32, name="cTe")
            cTo = pp.tile([P, CH, 16], F32, name="cTo")
            oTe = pp.tile([P, CH, 16], F32, name="oTe")
            oTo = pp.tile([P, CH, 16], F32, name="oTo")
            # h split into K-half tiles, double-buffered by step parity
            hb = [[pp.tile([P, 8], F16, name=f"h{kt}{s}") for s in range(2)]
                  for kt in range(2)]
            warm = pp.tile([P, 1], F32, name="warm")

            nc.sync.dma_start(WXs[:], wx[:].rearrange("kt mt k m -> k kt mt m"))
            nc.sync.dma_start(WHs[:], wh[:].rearrange("kt mt k m -> k kt mt m"))
            nc.sync.dma_start(BS[:], bcol[:].rearrange("mt k -> k mt"))
            for kt in range(2):
                nc.vector.memset(hb[kt][0][:], 0.0)
            nc.vector.memset(cTo[:], 0.0)
            nc.vector.memset(warm[:], 0.0)
            # Load the sigmoid/tanh table set before the loop (hoistable).
            nc.scalar.activation(warm[:], warm[:], AF.Sigmoid)
            nc.scalar.activation(warm[:], warm[:], AF.Tanh)
            tc.strict_bb_all_engine_barrier()

            # ---- helpers ----
            def dma_x_chunk(t0_sv):
                """DMA one chunk of pre-transposed x: [128, 2, 256]."""
                xTs = xp.tile([P, 2, UCOLS], XDT, tag="xTs")
                src = xT_d[:, :, :, :][:, :, ds(t0_sv, CH), :]
                nc.sync.dma_start(xTs[:], src.rearrange("kt e t b -> e kt t b"))
                return xTs

            def u_piece(xTs, U, mt, part):
                """Half of one M-tile of the U-chunk matmul (+ bias copy)."""
                if part == 0:
                    psu = up.tile([P, UCOLS], F32, tag="upsum", name=f"psu{mt}")
                    u_psu[mt % 2] = psu
                    nc.tensor.matmul(psu[:], WXs[:, 0, mt, :], xTs[:, 0, :],
                                     start=True, stop=False)
                else:
                    psu = u_psu[mt % 2]
                    nc.tensor.matmul(psu[:], WXs[:, 1, mt, :], xTs[:, 1, :],
                                     start=False, stop=True)
                    nc.vector.tensor_scalar(
                        out=U[:, mt, :], in0=psu[:], scalar1=BS[:, mt:mt + 1],
                        scalar2=None, op0=ALU.add)

            u_psu = [None, None]

            def step(t, U, cT, cT_prev_buf):
                """One LSTM timestep (t = index within chunk)."""
                h_prev = [hb[0][t % 2], hb[1][t % 2]]
                h_new = [hb[0][1 - t % 2], hb[1][1 - t % 2]]
                c_prev = (cT[:, t - 1, :] if t > 0
                          else cT_prev_buf[:, CH - 1, :])
                c_prev = c_prev.rearrange("p (h b) -> p h b", h=2)

                # i/j/f m-tiles (0..5) in one psum tile, o (6..7) in another
                zpa = zp.tile([P, 48], F32, tag="zpa")
                zpo = zp.tile([P, 16], F32, tag="zpo")
                for mt in range(6):
                    for kt in range(2):
                        nc.tensor.matmul(
                            zpa[:, mt * 8:(mt + 1) * 8],
                            WHs[:, kt, mt, :], h_prev[kt][:],
                            start=(kt == 0), stop=(kt == 1))
                for mt in range(6, 8):
                    for kt in range(2):
                        nc.tensor.matmul(
                            zpo[:, (mt - 6) * 8:(mt - 5) * 8],
                            WHs[:, kt, mt, :], h_prev[kt][:],
                            start=(kt == 0), stop=(kt == 1))

                zba = sp.tile([P, 6, 8], F32, tag="zba")
                nc.vector.tensor_tensor(
                    zba[:], zpa[:].rearrange("p (m b) -> p m b", b=8),
                    U[:, 0:6, t * 8:(t + 1) * 8], ALU.add)
                sga = sp.tile([P, 6, 8], F32, tag="sga")
                nc.scalar.activation(sga[:], zba[:], AF.Sigmoid)

                zbo = sp.tile([P, 2, 8], F32, tag="zbo")
                nc.vector.tensor_tensor(
                    zbo[:], zpo[:].rearrange("p (m b) -> p m b", b=8),
                    U[:, 6:8, t * 8:(t + 1) * 8], ALU.add)
                sgo = sp.tile([P, 2, 8], F32, tag="sgo")
                nc.scalar.activation(sgo[:], zbo[:], AF.Sigmoid)

                gi, gj, gf = sga[:, 0:2], sga[:, 2:4], sga[:, 4:6]

                t1 = sp.tile([P, 2, 8], F32, tag="t1")
                nc.vector.tensor_mul(t1[:], gi, gj)
                d = sp.tile([P, 2, 8], F32, tag="d")
                nc.vector.scalar_tensor_tensor(
                    d[:], t1[:], 2.0, gi, ALU.mult, ALU.subtract)
                ca = sp.tile([P, 2, 8], F32, tag="ca")
                nc.gpsimd.tensor_mul(ca[:], c_prev, gf)
                cw = cT[:, t, :].rearrange("p (h b) -> p h b", h=2)
                nc.vector.tensor_add(cw, ca[:], d[:])

                th = sp.tile([P, 2, 8], F32, tag="th")
                th_i = nc.scalar.activation(th[:], cw, AF.Tanh)
                # prime the PE pipeline while h is being produced: a dummy
                # matmul gated on the tanh so it fires just before the real
                # burst and absorbs the SBUF-access pipeline-fill latency
                dps = zp.tile([P, 8], F32, tag="dps")
                dmm = nc.tensor.matmul(dps[:, 0:1], WHs[:, 0, 0, :],
                                       h_prev[0][:, 0:1],
                                       start=True, stop=True)
                tile.add_dep_helper(dmm.ins, th_i.ins, True, "pe primer")
                for kt in range(2):
                    nc.vector.tensor_mul(h_new[kt][:], th[:, kt], sgo[:, kt])
                    nc.gpsimd.tensor_copy(
                        oT_cur[0][:, t, kt * 8:(kt + 1) * 8], h_new[kt][:])

            oT_cur = [None]

            def run_chunk(U, cT, cT_prev, oT, extras):
                oT_cur[0] = oT
                for t in range(CH):
                    step(t, U, cT, cT_prev)
                    for fn in extras.get(t, ()):
                        fn()

            def build_u_extras(xTs_box, U, extras, store=None):
                """Spread the 16 U matmul halves + copies over the steps."""
                for mt in range(8):
                    for part in range(2):
                        extras.setdefault(3 + mt * 2 + part, []).append(
                            lambda mt=mt, part=part:
                                u_piece(xTs_box[0], U, mt, part))
                if store is not None:
                    extras.setdefault(22, []).append(store)

            # ---- prologue: build U for chunk 0 ----
            xTs0 = dma_x_chunk(0)
            for mt in range(8):
                u_piece(xTs0, UA, mt, 0)
                u_piece(xTs0, UA, mt, 1)

            # ---- main loop: one iteration = chunks (2i, 2i+1) ----
            with tc.For_i(0, ITERS, 1,
                          hint_engines=(mybir.EngineType.PE,
                                        mybir.EngineType.DVE)) as it:
                t0o = it * (2 * CH) + CH
                t0n = it * (2 * CH) + 2 * CH

                # even chunk; build UB (for the odd chunk) alongside
                boxo = [None]
                extras = {1: [lambda: boxo.__setitem__(0, dma_x_chunk(t0o))]}
                build_u_extras(boxo, UB, extras)
                run_chunk(UA, cTe, cTo, oTe, extras)

                # odd chunk; build UA for next iteration's even chunk and
                # store the even chunk's histories alongside
                boxn = [None]
                extras = {1: [lambda: boxn.__setitem__(0, dma_x_chunk(t0n))]}

                def store_even(it=it):
                    nc.sync.dma_start(
                        o_raw[ds(it * 2, 1)],
                        oTe[:].rearrange("p t c -> p (t c)"))
                    nc.sync.dma_start(
                        c_raw[ds(it * 2, 1)],
                        cTe[:].rearrange("p t c -> p (t c)"))

                build_u_extras(boxn, UA, extras, store=store_even)
                run_chunk(UB, cTo, cTe, oTo, extras)

                nc.sync.dma_start(
                    o_raw[ds(it * 2 + 1, 1)],
                    oTo[:].rearrange("p t c -> p (t c)"))
                nc.sync.dma_start(
                    c_raw[ds(it * 2 + 1, 1)],
                    cTo[:].rearrange("p t c -> p (t c)"))

    return nc


def _prep_weights(W, b):
    """Host-side weight prep: split/scale/reorder into device layouts."""
    W = np.asarray(W, np.float32)
    b = np.asarray(b, np.float32)
    Wx = W[:E].copy()            # [256, 1024]
    Wh = W[E:].copy()            # [256, 1024]
    b_eff = b.copy()
    # j gate (cols 256:512) pre-scaled by 2 for tanh(j) = 2*sigmoid(2j) - 1
    Wx[:, 256:512] *= 2.0
    Wh[:, 256:512] *= 2.0
    b_eff[256:512] *= 2.0
    # f gate (cols 512:768): reference uses sigmoid(f + 1)
    b_eff[512:768] += 1.0

    def to_tiles(M, dt):
        a = M.reshape(2, P, 8, P).transpose(0, 2, 1, 3)
        return np.ascontiguousarray(a).astype(dt)

    wx_t = to_tiles(Wx, np.float32)
    wh_t = to_tiles(Wh, np.float16)
    bcol = np.ascontiguousarray(b_eff.reshape(8, P)).astype(np.float32)
    return wx_t, wh_t, bcol


def kernel(input_embeddings, input_length, W, b):
    from concourse.bass_utils import run_bass_kernel_spmd

    x = np.asarray(input_embeddings, np.float32)
    lengths = np.asarray(input_length, np.int32)
    B, T, _ = x.shape
    assert B == N_CORES * B_LOC

    key = T
    if key not in _CACHE:
        nc_new = build_nc(T)
        if not nc_new.is_finalized():
            nc_new.finalize()
        _CACHE[key] = nc_new
    nc = _CACHE[key]

    wx_t, wh_t, bcol = _prep_weights(W, b)
    TPAD = T + 2 * CH

    # pre-transpose x per core: [2(kt), 128(e), TPAD, 8(b)]
    in_maps = []
    for k in range(N_CORES):
        xs = x[k * B_LOC:(k + 1) * B_LOC]              # [8, T, 256]
        xt = np.zeros((2, P, TPAD, B_LOC), np.float32)
        xt[:, :, :T] = xs.transpose(2, 1, 0).reshape(2, P, T, B_LOC)
        in_maps.append({"xT": xt, "wx": wx_t, "wh": wh_t, "bcol": bcol})

    trace = os.environ.get("KERNEL_PROFILE", "0") == "1"
    kw = {}
    if trace:
        try:
            import profshim
            profshim.install()
            kw["tmpdir"] = os.environ.get("KERNEL_TRACE_DIR") or None
        except Exception:
            trace = False
    try:
        res = run_bass_kernel_spmd(nc, in_maps, list(range(N_CORES)),
                                   trace=trace, **kw)
    except ModuleNotFoundError:
        res = run_bass_kernel_spmd(nc, in_maps, list(range(N_CORES)), trace=False)
    global LAST_EXEC_NS
    LAST_EXEC_NS = res.exec_time_ns

    NCH = T // CH
    outputs = np.empty((B, T, H), np.float32)
    c_fin = np.zeros((B, H), np.float32)
    h_fin = np.zeros((B, H), np.float32)
    for k in range(N_CORES):
        # o_raw [NCH, 128, CH*16], col = t*16 + half*8 + b
        o = np.asarray(res.results[k]["o_raw"]).reshape(NCH, P, CH, 2, B_LOC)
        # -> outputs[b, t, half*128 + p]
        o = o.transpose(4, 0, 2, 3, 1).reshape(B_LOC, T, H)
        outputs[k * B_LOC:(k + 1) * B_LOC] = o

    for bi in range(B):
        L = int(lengths[bi])
        if L > 0:
            h_fin[bi] = outputs[bi, L - 1]
            core, bl = bi // B_LOC, bi % B_LOC
            craw = res.results[core]["c_raw"]  # [NCH, 128, CH*16]
            ci, tr = (L - 1) // CH, (L - 1) % CH
            for half in range(2):
                c_fin[bi, half * P:(half + 1) * P] = \
                    craw[ci, :, tr * 16 + half * 8 + bl]
        outputs[bi, L:] = 0.0

    return outputs, (h_fin, c_fin)


LAST_EXEC_NS = None


# revision 28
# speedup vs baseline: 1.2782x; 1.0143x over previous
"""Trainium2 Bass kernel for a ragged-length LSTM encoder.

Problem: B=64, T=2048, E=H=256 LSTM with per-row lengths; outputs [B,T,H]
(zeros past each row's length) plus final (h, c) state frozen at length-1.

Strategy (data-parallel, 8 rows per NeuronCore; latency-optimized serial
scan):
  - Transposed layout on chip: hidden dim on partitions, batch on free dim.
  - Recurrent matmul z_h^T = W_h^T @ h^T as 16 [128,128]x[128,8] matmuls
    per step with fp16 weights (fast weight load). i/j/f gate tiles land in
    one PSUM tile, o in another, so sigmoid(i,j,f) can fire after 12 MMs.
  - Input projection U = x @ W_x + b_eff precomputed on-device per 32-step
    chunk from a HOST-pre-transposed x (float32r matmuls, off critical
    path, double buffered).
  - tanh(j) = 2*sigmoid(2j) - 1 with j columns of W/b pre-scaled by 2.
  - h is produced as two fp16 half-tiles so the next step's K0 matmuls
    start before the second half is ready; a dummy matmul primes the PE
    pipeline right after tanh.
  - No masking/freezing on device; h and c histories are DMA'd out raw
    (t-major) and the host transposes/assembles outputs, zeroes tails,
    and gathers final h and c.
"""

import os
import numpy as np

import concourse.bass as bass
import concourse.mybir as mybir
import concourse.tile as tile
from concourse import bacc
from concourse.bass import ds

AF = mybir.ActivationFunctionType
ALU = mybir.AluOpType
F32 = mybir.dt.float32
F32R = mybir.dt.float32r
F16 = mybir.dt.float16

P = 128
B_LOC = 8       # batch rows per core
E = 256         # input embedding dim
H = 256         # hidden dim
CH = 32         # timesteps per chunk
N_CORES = 8

_CACHE = {}


def build_nc(T, use_f32r=True):
    NCH = T // CH           # number of chunks
    ITERS = NCH // 2        # For_i iterations (2 chunks per body)
    TPAD = T + 2 * CH       # xT is padded so next-chunk prefetch never OOBs
    UCOLS = B_LOC * CH      # 256 cols per U chunk, t-major: col = t*8 + b
    XDT = F32R if use_f32r else F32

    nc = bacc.Bacc(trn_type="TRN2")

    # host-pre-transposed x: [kt(e-half), e(128), t, b]
    xT_d = nc.dram_tensor("xT", [2, P, TPAD, B_LOC], XDT, kind="ExternalInput")
    wx = nc.dram_tensor("wx", [2, 8, P, P], XDT, kind="ExternalInput")
    wh = nc.dram_tensor("wh", [2, 8, P, P], F16, kind="ExternalInput")
    bcol = nc.dram_tensor("bcol", [8, P], F32, kind="ExternalInput")

    # raw histories; host reassembles. col = t*16 + half*8 + b
    o_raw = nc.dram_tensor("o_raw", [NCH, P, 16 * CH], F32,
                           kind="ExternalOutput")
    c_raw = nc.dram_tensor("c_raw", [NCH, P, 16 * CH], F32,
                           kind="ExternalOutput")

    with tile.TileContext(nc) as tc:
        with (
            tc.tile_pool(name="persist", bufs=1) as pp,
            tc.tile_pool(name="step", bufs=3) as sp,
            tc.tile_pool(name="xs", bufs=2) as xp,
            tc.tile_pool(name="zpsum", bufs=2, space="PSUM") as zp,
            tc.tile_pool(name="upsum", bufs=2, space="PSUM") as up,
        ):
            # ---- persistent tiles ----
            WXs = pp.tile([P, 2, 8, P], XDT, name="WXs")
            WHs = pp.tile([P, 2, 8, P], F16, name="WHs")
            BS = pp.tile([P, 8], F32, name="BS")
            xS = [pp.tile([P, 2, UCOLS], XDT, name=f"xS{i}") for i in range(2)]
            UA = pp.tile([P, 8, UCOLS], F32, name="UA")
            UB = pp.tile([P, 8, UCOLS], F32, name="UB")
            cTe = pp.tile([P, CH, 16], F32, name="cTe")
            cTo = pp.tile([P, CH, 16], F32, name="cTo")
            oTe = pp.tile([P, CH, 16], F32, name="oTe")
            oTo = pp.tile([P, CH, 16], F32, name="oTo")
            # h split into K-half tiles, double-buffered by step parity
            hb = [[pp.tile([P, 8], F16, name=f"h{kt}{s}") for s in range(2)]
                  for kt in range(2)]
            warm = pp.tile([P, 1], F32, name="warm")

            nc.sync.dma_start(WXs[:], wx[:].rearrange("kt mt k m -> k kt mt m"))
            nc.sync.dma_start(WHs[:], wh[:].rearrange("kt mt k m -> k kt mt m"))
            nc.sync.dma_start(BS[:], bcol[:].rearrange("mt k -> k mt"))
            for kt in range(2):
                nc.vector.memset(hb[kt][0][:], 0.0)
            nc.vector.memset(cTo[:], 0.0)
            nc.vector.memset(warm[:], 0.0)
            # Load the sigmoid/tanh table set before the loop (hoistable).
            nc.scalar.activation(warm[:], warm[:], AF.Sigmoid)
            nc.scalar.activation(warm[:], warm[:], AF.Tanh)
            tc.strict_bb_all_engine_barrier()

            # ---- helpers ----
            def dma_x_chunk(slot, t0_sv):
                """DMA one chunk of pre-transposed x into xS[slot]."""
                src = xT_d[:, :, :, :][:, :, ds(t0_sv, CH), :]
                nc.sync.dma_start(xS[slot][:],
                                  src.rearrange("kt e t b -> e kt t b"))

            def u_piece(xTs, U, mt, part):
                """Quarter-pieces of one M-tile of the U-chunk matmul."""
                if part == 0:
                    psu = up.tile([P, UCOLS], F32, tag="upsum", name=f"psu{mt}")
                    u_psu[mt % 2] = psu
                    nc.tensor.matmul(psu[:], WXs[:, 0, mt, :], xTs[:, 0, :],
                                     start=True, stop=False)
                elif part == 1:
                    psu = u_psu[mt % 2]
                    nc.tensor.matmul(psu[:], WXs[:, 1, mt, :], xTs[:, 1, :],
                                     start=False, stop=True)
                else:
                    h0 = (part - 2) * P
                    psu = u_psu[mt % 2]
                    nc.vector.tensor_scalar(
                        out=U[:, mt, h0:h0 + P], in0=psu[:, h0:h0 + P],
                        scalar1=BS[:, mt:mt + 1], scalar2=None, op0=ALU.add)

            u_psu = [None, None]

            def step(t, U, cT, cT_prev_buf, extras=()):
                """One LSTM timestep (t = index within chunk)."""
                h_prev = [hb[0][t % 2], hb[1][t % 2]]
                h_new = [hb[0][1 - t % 2], hb[1][1 - t % 2]]
                c_prev = (cT[:, t - 1, :] if t > 0
                          else cT_prev_buf[:, CH - 1, :])
                c_prev = c_prev.rearrange("p (h b) -> p h b", h=2)

                # i/j/f m-tiles (0..5) in one psum tile, o (6..7) in another
                zpa = zp.tile([P, 48], F32, tag="zpa")
                zpo = zp.tile([P, 16], F32, tag="zpo")
                for mt in range(6):
                    for kt in range(2):
                        nc.tensor.matmul(
                            zpa[:, mt * 8:(mt + 1) * 8],
                            WHs[:, kt, mt, :], h_prev[kt][:],
                            start=(kt == 0), stop=(kt == 1))
                for mt in range(6, 8):
                    for kt in range(2):
                        nc.tensor.matmul(
                            zpo[:, (mt - 6) * 8:(mt - 5) * 8],
                            WHs[:, kt, mt, :], h_prev[kt][:],
                            start=(kt == 0), stop=(kt == 1))

                # dribbled off-path work goes right after the burst so it
                # never queues behind the tanh-gated primer on PE/DVE
                for fn in extras:
                    fn()

                zba = zp.tile([P, 48], F32, tag="zba")
                nc.vector.tensor_tensor(
                    zba[:].rearrange("p (m b) -> p m b", b=8),
                    zpa[:].rearrange("p (m b) -> p m b", b=8),
                    U[:, 0:6, t * 8:(t + 1) * 8], ALU.add)
                sga = sp.tile([P, 6, 8], F32, tag="sga")
                nc.scalar.activation(
                    sga[:], zba[:].rearrange("p (m b) -> p m b", b=8),
                    AF.Sigmoid)

                zbo = sp.tile([P, 2, 8], F32, tag="zbo")
                nc.vector.tensor_tensor(
                    zbo[:], zpo[:].rearrange("p (m b) -> p m b", b=8),
                    U[:, 6:8, t * 8:(t + 1) * 8], ALU.add)
                sgo = sp.tile([P, 2, 8], F32, tag="sgo")
                nc.scalar.activation(sgo[:], zbo[:], AF.Sigmoid)

                gi, gj, gf = sga[:, 0:2], sga[:, 2:4], sga[:, 4:6]

                t1 = sp.tile([P, 2, 8], F32, tag="t1")
                nc.vector.tensor_mul(t1[:], gi, gj)
                d = sp.tile([P, 2, 8], F32, tag="d")
                nc.vector.scalar_tensor_tensor(
                    d[:], t1[:], 2.0, gi, ALU.mult, ALU.subtract)
                ca = sp.tile([P, 2, 8], F32, tag="ca")
                nc.gpsimd.tensor_mul(ca[:], c_prev, gf)
                cw = cT[:, t, :].rearrange("p (h b) -> p h b", h=2)
                nc.vector.tensor_add(cw, ca[:], d[:])

                th = sp.tile([P, 2, 8], F32, tag="th")
                th_i = nc.scalar.activation(th[:], cw, AF.Tanh)
                # prime the PE pipeline while h is being produced: a dummy
                # matmul gated on the tanh so it fires just before the real
                # burst and absorbs the SBUF-access pipeline-fill latency
                dps = zp.tile([P, 16], F32, tag="zpo")
                dmm = nc.tensor.matmul(dps[:, 0:8], WHs[:, 0, 0, :],
                                       h_prev[0][:],
                                       start=True, stop=True)
                tile.add_dep_helper(dmm.ins, th_i.ins, True, "pe primer")
                for kt in range(2):
                    nc.vector.tensor_mul(h_new[kt][:], th[:, kt], sgo[:, kt])
                    nc.gpsimd.tensor_copy(
                        oT_cur[0][:, t, kt * 8:(kt + 1) * 8], h_new[kt][:])

            oT_cur = [None]

            def run_chunk(U, cT, cT_prev, oT, extras):
                oT_cur[0] = oT
                for t in range(CH):
                    step(t, U, cT, cT_prev, extras.get(t, ()))

            def build_u_extras(slot, U, extras, dma=None, store=None):
                """Spread the U matmul/copy pieces over the chunk's steps."""
                if dma is not None:
                    extras.setdefault(0, []).append(dma)
                for mt in range(8):
                    for part in range(4):
                        extras.setdefault(3 + mt * 2 + part, []).append(
                            lambda mt=mt, part=part:
                                u_piece(xS[slot], U, mt, part))
                if store is not None:
                    extras.setdefault(24, []).append(store)

            # ---- prologue: build U for chunk 0, prefetch chunk 1 ----
            dma_x_chunk(0, 0)
            for mt in range(8):
                for part in range(4):
                    u_piece(xS[0], UA, mt, part)
            dma_x_chunk(1, CH)

            # ---- main loop: one iteration = chunks (2i, 2i+1) ----
            with tc.For_i(0, ITERS, 1,
                          hint_engines=(mybir.EngineType.PE,
                                        mybir.EngineType.DVE)) as it:
                # even chunk: prefetch x for chunk 2i+2 into slot 0,
                # build UB (chunk 2i+1) from slot 1
                extras = {}
                build_u_extras(
                    1, UB, extras,
                    dma=lambda: dma_x_chunk(0, it * (2 * CH) + 2 * CH))
                run_chunk(UA, cTe, cTo, oTe, extras)

                # odd chunk: prefetch x for chunk 2i+3 into slot 1, build
                # UA (chunk 2i+2) from slot 0, store even-chunk histories
                def store_even(it=it):
                    nc.sync.dma_start(
                        o_raw[ds(it * 2, 1)],
                        oTe[:].rearrange("p t c -> p (t c)"))
                    nc.sync.dma_start(
                        c_raw[ds(it * 2, 1)],
                        cTe[:].rearrange("p t c -> p (t c)"))

                extras = {}
                build_u_extras(
                    0, UA, extras,
                    dma=lambda: dma_x_chunk(1, it * (2 * CH) + 3 * CH),
                    store=store_even)
                run_chunk(UB, cTo, cTe, oTo, extras)

                nc.sync.dma_start(
                    o_raw[ds(it * 2 + 1, 1)],
                    oTo[:].rearrange("p t c -> p (t c)"))
                nc.sync.dma_start(
                    c_raw[ds(it * 2 + 1, 1)],
                    cTo[:].rearrange("p t c -> p (t c)"))

    return nc


def _prep_weights(W, b):
    """Host-side weight prep: split/scale/reorder into device layouts."""
    W = np.asarray(W, np.float32)
    b = np.asarray(b, np.float32)
    Wx = W[:E].copy()            # [256, 1024]
    Wh = W[E:].copy()            # [256, 1024]
    b_eff = b.copy()
    # j gate (cols 256:512) pre-scaled by 2 for tanh(j) = 2*sigmoid(2j) - 1
    Wx[:, 256:512] *= 2.0
    Wh[:, 256:512] *= 2.0
    b_eff[256:512] *= 2.0
    # f gate (cols 512:768): reference uses sigmoid(f + 1)
    b_eff[512:768] += 1.0

    def to_tiles(M, dt):
        a = M.reshape(2, P, 8, P).transpose(0, 2, 1, 3)
        return np.ascontiguousarray(a).astype(dt)

    wx_t = to_tiles(Wx, np.float32)
    wh_t = to_tiles(Wh, np.float16)
    bcol = np.ascontiguousarray(b_eff.reshape(8, P)).astype(np.float32)
    return wx_t, wh_t, bcol


def kernel(input_embeddings, input_length, W, b):
    from concourse.bass_utils import run_bass_kernel_spmd

    x = np.asarray(input_embeddings, np.float32)
    lengths = np.asarray(input_length, np.int32)
    B, T, _ = x.shape
    assert B == N_CORES * B_LOC

    key = T
    if key not in _CACHE:
        nc_new = build_nc(T)
        if not nc_new.is_finalized():
            nc_new.finalize()
        _CACHE[key] = nc_new
    nc = _CACHE[key]

    wx_t, wh_t, bcol = _prep_weights(W, b)
    TPAD = T + 2 * CH

    # pre-transpose x per core: [2(kt), 128(e), TPAD, 8(b)]
    in_maps = []
    for k in range(N_CORES):
        xs = x[k * B_LOC:(k + 1) * B_LOC]              # [8, T, 256]
        xt = np.zeros((2, P, TPAD, B_LOC), np.float32)
        xt[:, :, :T] = xs.transpose(2, 1, 0).reshape(2, P, T, B_LOC)
        in_maps.append({"xT": xt, "wx": wx_t, "wh": wh_t, "bcol": bcol})

    trace = os.environ.get("KERNEL_PROFILE", "0") == "1"
    kw = {}
    if trace:
        try:
            import profshim
            profshim.install()
            kw["tmpdir"] = os.environ.get("KERNEL_TRACE_DIR") or None
        except Exception:
            trace = False
    try:
        res = run_bass_kernel_spmd(nc, in_maps, list(range(N_CORES)),
                                   trace=trace, **kw)
    except ModuleNotFoundError:
        res = run_bass_kernel_spmd(nc, in_maps, list(range(N_CORES)), trace=False)
    global LAST_EXEC_NS
    LAST_EXEC_NS = res.exec_time_ns

    NCH = T // CH
    outputs = np.empty((B, T, H), np.float32)
    c_fin = np.zeros((B, H), np.float32)
    h_fin = np.zeros((B, H), np.float32)
    for k in range(N_CORES):
        # o_raw [NCH, 128, CH*16], col = t*16 + half*8 + b
        o = np.asarray(res.results[k]["o_raw"]).reshape(NCH, P, CH, 2, B_LOC)
        # -> outputs[b, t, half*128 + p]
        o = o.transpose(4, 0, 2, 3, 1).reshape(B_LOC, T, H)
        outputs[k * B_LOC:(k + 1) * B_LOC] = o

    for bi in range(B):
        L = int(lengths[bi])
        if L > 0:
            h_fin[bi] = outputs[bi, L - 1]
            core, bl = bi // B_LOC, bi % B_LOC
            craw = res.results[core]["c_raw"]  # [NCH, 128, CH*16]
            ci, tr = (L - 1) // CH, (L - 1) % CH
            for half in range(2):
                c_fin[bi, half * P:(half + 1) * P] = \
                    craw[ci, :, tr * 16 + half * 8 + bl]
        outputs[bi, L:] = 0.0

    return outputs, (h_fin, c_fin)


LAST_EXEC_NS = None
